# revision 10
# baseline (speedup 1.0000x reference)
"""Two-layer DGL-style GraphConv (norm='both') on 8 Trainium2 NeuronCores.

Strategy (dst-sharded message passing, bf16 quarter-major tables,
pipelined collectives):
  - Host: two-phase balance of nodes into 8 shards x nb blocks of 128
    (phase 1 equalizes cell in-degree and fixes each node's table quarter;
    phase 2 re-packs nodes within each quarter to equalize per-(cell,
    src-quarter) edge counts, minimizing 128-edge chunk padding).
    Both gather tables (y and z) use the same quarter-major layout:
    table row (within quarter q) = core*128*nbq + loc*nbq + block_in_q,
    which makes each AllGather chunk a single contiguous partition-major
    DMA from SBUF staging (128 large descriptors instead of thousands of
    512B ones). Edges classified per (dst core, dst block, src quarter),
    padded to 128-edge chunks; one shared int16 gather index table
    (wrapped layout) + per-edge dst keys.
  - Device (SPMD, identical program on 8 cores, per-core data, bf16):
      phase 1: y = ns * (x @ W1) per block into SBUF staging; flush +
        AllGather per 13-block quarter (4 pipelined collectives).
      L1: four src-quarter passes over all dst blocks; pass q scatters
        quarter-q chunks (dma_gather 512B rows + one-hot matmuls in PSUM);
        passes 0-2 accumulate into an SBUF partial, pass 3 finishes:
        h = relu((psum+partial)*nd + b1); z = ns * (h @ W2) into staging;
        flush + AllGather z per quarter (4 pipelined collectives).
      L2: same four-pass message passing on z; out = (psum+partial)*nd+b2
        accumulated in SBUF, single flush at the end.
  - Host: inverse-permute output shards.
"""

import sys

sys.path.insert(0, "/opt/trn_rl_repo")

import ml_dtypes
import numpy as np

import concourse.bass as bass
import concourse.mybir as mybir
import concourse.tile as tile
from concourse import bacc
from concourse.bass_utils import run_bass_kernel_spmd
from concourse.masks import make_identity

F32 = mybir.dt.float32
BF16 = mybir.dt.bfloat16
I16 = mybir.dt.int16
NPBF16 = ml_dtypes.bfloat16

NCORES = 8
NQ = 4  # src-quarter streams


# ----------------------------------------------------------------------------
# Host-side preprocessing
# ----------------------------------------------------------------------------

def _balance_nodes(deg_in, ncores, nb):
    """Phase-1 assignment: nodes to ncores*nb cells of <=128, equalizing cell
    in-degree (greedy heaviest-first). Returns cell index per node."""
    import heapq

    N = deg_in.shape[0]
    ncells = ncores * nb
    order = np.argsort(-deg_in, kind="stable")
    heap = [(0.0, c) for c in range(ncells)]
    heapq.heapify(heap)
    cnt = np.zeros(ncells, dtype=np.int64)
    cell_of = np.empty(N, dtype=np.int64)
    for node in order:
        while True:
            load, c = heapq.heappop(heap)
            if cnt[c] < 128:
                break
        cell_of[node] = c
        cnt[c] += 1
        heapq.heappush(heap, (load + float(deg_in[node]), c))
    return cell_of


def _repack_quarter(dloads, ncells):
    """Phase-2: re-pack nodes (rows of dloads [n, NQ]) into ncells cells
    (cap 128), minimizing the max per-cell load across all NQ src-quarter
    dimensions (multi-dim LPT greedy). Returns cell index per node."""
    n = dloads.shape[0]
    key = dloads.max(axis=1) + 1e-3 * dloads.sum(axis=1)
    order = np.argsort(-key, kind="stable")
    L = np.zeros((ncells, NQ))
    cnt = np.zeros(ncells, dtype=np.int64)
    out = np.empty(n, dtype=np.int64)
    BIG = 1e18
    for i in order:
        cost = (L + dloads[i]).max(axis=1) + np.where(cnt >= 128, BIG, 0.0)
        c = int(np.argmin(cost))
        out[i] = c
        L[c] += dloads[i]
        cnt[c] += 1
    return out


def prepare(x, W1, b1, W2, b2, src, dst, ncores=NCORES):
    """Host preprocessing. Returns (in_maps, meta)."""
    N, IN = x.shape
    HID = W1.shape[1]
    OUT = W2.shape[1]
    nbq = -(-N // (128 * ncores * NQ))  # blocks per quarter per core
    nb = NQ * nbq
    NPC = nb * 128
    NPQ = nbq * 128
    NTOT = ncores * NPC
    QTOT = ncores * NPQ
    assert QTOT <= 32768, "int16 gather index limit"

    deg_out = np.bincount(src, minlength=N).astype(np.float32)
    deg_in = np.bincount(dst, minlength=N).astype(np.float32)
    ns_full = 1.0 / np.sqrt(np.maximum(deg_out, 1.0))
    nd_full = 1.0 / np.sqrt(np.maximum(deg_in, 1.0))

    # ---- phase 1: fixes each node's quarter ----
    cell1 = _balance_nodes(deg_in, ncores, nb)
    quarter = (cell1 % nb) // nbq

    # per-node in-edge split by src quarter
    esq = quarter[src]
    dloads = np.stack(
        [np.bincount(dst[esq == q], minlength=N).astype(np.float64)
         for q in range(NQ)],
        axis=1,
    )

    # ---- phase 2: re-pack within each quarter ----
    c_n = np.empty(N, dtype=np.int64)
    b_n = np.empty(N, dtype=np.int64)
    loc_n = np.empty(N, dtype=np.int64)
    for q in range(NQ):
        nodes = np.nonzero(quarter == q)[0]
        cells = _repack_quarter(dloads[nodes], ncores * nbq)
        cc = cells // nbq
        bb = cells % nbq + q * nbq
        c_n[nodes] = cc
        b_n[nodes] = bb
        ordc = np.lexsort((nodes, cc * nb + bb))
        sorted_cell = (cc * nb + bb)[ordc]
        loc = np.arange(len(nodes)) - np.searchsorted(
            sorted_cell, sorted_cell, side="left"
        )
        loc_n[nodes[ordc]] = loc
    assert loc_n.max() < 128

    pos = c_n * NPC + b_n * 128 + loc_n      # bounce-order position
    q_n = b_n // nbq
    bq_n = b_n - q_n * nbq                   # block within quarter
    # table row: quarter-major, partition-major within each core section
    tslot = q_n * QTOT + c_n * NPQ + loc_n * nbq + bq_n
    trel = c_n * NPQ + loc_n * nbq + bq_n    # quarter-relative row

    # edge classification
    e_core = c_n[dst]
    e_block = b_n[dst]
    e_dloc = loc_n[dst]
    e_q = q_n[src]
    e_idx = trel[src].astype(np.int64)

    order = np.lexsort((e_idx, e_q, e_block, e_core))
    e_core = e_core[order]
    e_block = e_block[order]
    e_dloc = e_dloc[order]
    e_q = e_q[order]
    e_idx = e_idx[order]

    cell = (e_core * nb + e_block) * NQ + e_q
    counts = np.bincount(cell, minlength=ncores * nb * NQ).reshape(
        ncores, nb, NQ
    )
    C = np.maximum(1, -(-counts.max(axis=0) // 128))  # chunks per (b, q)
    cmax = int(C.max())
    chunk_off = np.zeros((nb, NQ), dtype=np.int64)
    q_ch0 = [0] * NQ
    q_nch = [0] * NQ
    acc = 0
    for s in range(NQ):
        q_ch0[s] = acc
        for b in range(nb):
            chunk_off[b, s] = acc
            acc += int(C[b, s])
        q_nch[s] = acc - q_ch0[s]
    nch = acc
    nslot = nch * 128

    flat_counts = counts.reshape(-1)
    cell_starts = np.concatenate([[0], np.cumsum(flat_counts)[:-1]]).reshape(
        ncores, nb, NQ
    )

    idx_slots = np.zeros((ncores, nslot), dtype=np.int16)
    dk_slots = np.full((ncores, nslot), 999.0, dtype=np.float32)  # ->1000 in bf16, never matches iota 0..127
    for c in range(ncores):
        for b in range(nb):
            for s in range(NQ):
                cnt = int(counts[c, b, s])
                st = int(cell_starts[c, b, s])
                sl0 = int(chunk_off[b, s]) * 128
                idx_slots[c, sl0 : sl0 + cnt] = e_idx[st : st + cnt]
                dk_slots[c, sl0 : sl0 + cnt] = e_dloc[st : st + cnt]

    # wrapped int16 index layout: slot j -> [j%16, j//16], replicated x8
    idx_w = idx_slots.reshape(ncores, nslot // 16, 16).transpose(0, 2, 1)
    idx_w = np.ascontiguousarray(np.tile(idx_w, (1, 8, 1)))
    dk_w = np.ascontiguousarray(
        dk_slots.reshape(ncores, nch, 128).transpose(0, 2, 1)
    ).astype(NPBF16)

    ns_pad = np.zeros(NTOT, dtype=np.float32)
    nd_pad = np.ones(NTOT, dtype=np.float32)
    ns_pad[pos] = ns_full
    nd_pad[pos] = nd_full

    x_pad = np.zeros((NTOT, IN), dtype=NPBF16)
    x_pad[pos] = x.astype(NPBF16)

    iota = np.ascontiguousarray(
        np.tile(np.arange(128, dtype=np.float32)[None, None, :], (128, cmax, 1))
    ).astype(NPBF16)
    b1rep = np.ascontiguousarray(np.tile(b1.reshape(1, HID), (128, 1))).astype(
        np.float32
    )
    b2rep = np.ascontiguousarray(np.tile(b2.reshape(1, OUT), (128, 1))).astype(
        np.float32
    )

    KIN = IN // 128
    in_maps = []
    for c in range(ncores):
        lo, hi = c * NPC, (c + 1) * NPC
        in_maps.append(
            {
                "xT": np.ascontiguousarray(
                    x_pad[lo:hi].T.reshape(KIN, 128, NPC)
                ),
                "w1": np.ascontiguousarray(W1.astype(NPBF16)),
                "w2": np.ascontiguousarray(W2.astype(NPBF16)),
                "b1rep": b1rep,
                "b2rep": b2rep,
                "ns": np.ascontiguousarray(ns_pad[lo:hi].reshape(nb, 128).T),
                "nd": np.ascontiguousarray(nd_pad[lo:hi].reshape(nb, 128).T),
                "gidx": idx_w[c],
                "dkey": dk_w[c],
                "iota3": iota,
            }
        )

    meta = dict(
        ncores=ncores,
        N=N,
        IN=IN,
        HID=HID,
        OUT=OUT,
        nb=nb,
        nbq=nbq,
        NPC=NPC,
        NPQ=NPQ,
        NTOT=NTOT,
        QTOT=QTOT,
        C=C,
        chunk_off=chunk_off,
        q_ch0=q_ch0,
        q_nch=q_nch,
        nch=nch,
        nslot=nslot,
        cmax=cmax,
        pos=pos,
    )
    return in_maps, meta


def emulate(in_maps, meta, W1, b1, W2, b2):
    """Numpy emulation of the device program (for host-side index checking)."""
    ncores = meta["ncores"]
    nb, nbq = meta["nb"], meta["nbq"]
    NPC, NPQ, NTOT, QTOT = meta["NPC"], meta["NPQ"], meta["NTOT"], meta["QTOT"]
    C, chunk_off = meta["C"], meta["chunk_off"]
    HID, OUT = meta["HID"], meta["OUT"]

    def ag_table(stages, feat):
        # stages: per core list of [128, nbq, feat] per quarter
        tab = np.zeros((NTOT, feat), dtype=np.float32)
        for q in range(NQ):
            for c in range(ncores):
                blob = stages[c][q]  # [128, nbq, feat]
                rows = blob.reshape(128 * nbq, feat)  # row = p*nbq + b
                tab[q * QTOT + c * NPQ : q * QTOT + (c + 1) * NPQ] = rows
        return tab

    def mp(table, feat, c):
        m = in_maps[c]
        gidx = m["gidx"]  # [128, nslot//16]
        dkey = m["dkey"]  # [128, nch]
        # de-wrap idx: slot j -> gidx[j%16, j//16]
        nslot = meta["nslot"]
        idx = np.empty(nslot, dtype=np.int64)
        for j16 in range(16):
            idx[j16::16] = gidx[j16, :].astype(np.int64)[: nslot // 16]
        agg = np.zeros((nb, 128, feat), dtype=np.float32)
        for s in range(NQ):
            base = s * QTOT
            for b in range(nb):
                for ci in range(int(C[b, s])):
                    ch = int(chunk_off[b, s]) + ci
                    sl = slice(ch * 128, (ch + 1) * 128)
                    rows = table[base + idx[sl]]  # [128, feat]
                    keys = dkey[:, ch]  # [128] dst loc per slot
                    for p in range(128):
                        k = int(keys[p])
                        if k < 128:
                            agg[b, k] += rows[p]
        return agg

    x_stages = []
    for c in range(ncores):
        m = in_maps[c]
        xT = m["xT"].astype(np.float32)  # [KIN, 128, NPC]
        xc = xT.transpose(2, 0, 1).reshape(NPC, -1)  # [NPC, IN]
        y = (xc @ m["w1"].astype(np.float32)) * m["ns"].T.reshape(NPC, 1)
        y = y.astype(NPBF16).astype(np.float32)
        stg = [
            y[q * NPQ : (q + 1) * NPQ].reshape(nbq, 128, HID).transpose(1, 0, 2)
            for q in range(NQ)
        ]
        x_stages.append(stg)
    y_tab = ag_table(x_stages, HID)

    z_stages = []
    outs = []
    for c in range(ncores):
        m = in_maps[c]
        agg = mp(y_tab, HID, c).reshape(NPC, HID)
        h = np.maximum(
            agg * m["nd"].T.reshape(NPC, 1) + b1.reshape(1, HID), 0.0
        ).astype(NPBF16).astype(np.float32)
        z = (h @ m["w2"].astype(np.float32)) * m["ns"].T.reshape(NPC, 1)
        z = z.astype(NPBF16).astype(np.float32)
        z_stages.append([
            z[q * NPQ : (q + 1) * NPQ].reshape(nbq, 128, OUT).transpose(1, 0, 2)
            for q in range(NQ)
        ])
    z_tab = ag_table(z_stages, OUT)
    for c in range(ncores):
        m = in_maps[c]
        agg = mp(z_tab, OUT, c).reshape(NPC, OUT)
        o = agg * m["nd"].T.reshape(NPC, 1) + b2.reshape(1, OUT)
        outs.append(o)
    full = np.concatenate(outs, axis=0)
    return full[meta["pos"]]


# ----------------------------------------------------------------------------
# Bass program
# ----------------------------------------------------------------------------

def build_nc(meta):
    ncores = meta["ncores"]
    IN, HID, OUT = meta["IN"], meta["HID"], meta["OUT"]
    nb, nbq = meta["nb"], meta["nbq"]
    NPC, NPQ, NTOT, QTOT = meta["NPC"], meta["NPQ"], meta["NTOT"], meta["QTOT"]
    C, chunk_off, nch, nslot, cmax = (
        meta["C"],
        meta["chunk_off"],
        meta["nch"],
        meta["nslot"],
        meta["cmax"],
    )
    q_ch0, q_nch = meta["q_ch0"], meta["q_nch"]
    KIN = IN // 128
    KH = HID // 128

    nc = bacc.Bacc(
        "TRN2",
        target_bir_lowering=False,
        debug=False,
        num_devices=ncores,
        num_swdge_queues=4,
        dynamic_dma_scratch_size=32768,
    )

    xT = nc.dram_tensor("xT", [KIN, 128, NPC], BF16, kind="ExternalInput")
    w1 = nc.dram_tensor("w1", [IN, HID], BF16, kind="ExternalInput")
    w2 = nc.dram_tensor("w2", [HID, OUT], BF16, kind="ExternalInput")
    b1rep = nc.dram_tensor("b1rep", [128, HID], F32, kind="ExternalInput")
    b2rep = nc.dram_tensor("b2rep", [128, OUT], F32, kind="ExternalInput")
    ns = nc.dram_tensor("ns", [128, nb], F32, kind="ExternalInput")
    nd = nc.dram_tensor("nd", [128, nb], F32, kind="ExternalInput")
    gidx = nc.dram_tensor("gidx", [128, nslot // 16], I16, kind="ExternalInput")
    dkey = nc.dram_tensor("dkey", [128, nch], BF16, kind="ExternalInput")
    iota3 = nc.dram_tensor("iota3", [128, cmax, 128], BF16, kind="ExternalInput")
    out_t = nc.dram_tensor("out", [128, nb, OUT], F32, kind="ExternalOutput")

    y_bounce = nc.dram_tensor("y_bounce", [NQ, 128, nbq * HID], BF16)
    y_full = nc.dram_tensor("y_full", [NTOT, HID], BF16, addr_space="Shared")
    z_bounce = nc.dram_tensor("z_bounce", [NQ, 128, nbq * OUT], BF16)
    z_full = nc.dram_tensor("z_full", [NTOT, OUT], BF16, addr_space="Shared")

    groups = [list(range(ncores))]

    def allgather(src_ap, dst_ap):
        nc.gpsimd.collective_compute(
            "AllGather",
            mybir.AluOpType.bypass,
            replica_groups=groups,
            ins=[src_ap],
            outs=[dst_ap],
        )

    with tile.TileContext(nc) as tc:
        with (
            tc.tile_pool(name="const", bufs=1) as const_pool,
            tc.tile_pool(name="work", bufs=3) as work_pool,
            tc.tile_pool(name="stage", bufs=2) as stage_pool,
            tc.tile_pool(name="part", bufs=1) as part_pool,
            tc.tile_pool(name="psA", bufs=3, space="PSUM") as psA,
            tc.tile_pool(name="psB", bufs=2, space="PSUM") as psB,
            tc.tile_pool(name="psC", bufs=2, space="PSUM") as psC,
        ):
            # ---- phase-1-critical loads first ----
            w1_sb = const_pool.tile([128, KIN, HID], BF16)
            nc.sync.dma_start(
                w1_sb[:], w1[:].rearrange("(kt p) h -> p kt h", p=128)
            )
            ns_sb = const_pool.tile([128, nb], F32)
            nc.sync.dma_start(ns_sb[:], ns[:])

            with tc.tile_pool(name="xt", bufs=1) as xt_pool:
                xt_sb = []
                for kt in range(KIN):
                    t = xt_pool.tile([128, NPC], BF16, tag=f"xt{kt}")
                    nc.sync.dma_start(t[:, :NPQ], xT[kt, :, :NPQ])
                    xt_sb.append(t)
                for kt in range(KIN):
                    nc.sync.dma_start(xt_sb[kt][:, NPQ:], xT[kt, :, NPQ:])

                emitted_consts = [False]

                def late_consts():
                    # loads overlapped with the first y AllGather
                    emitted_consts[0] = True
                    w2_sb = const_pool.tile([128, KH, OUT], BF16)
                    nc.sync.dma_start(
                        w2_sb[:], w2[:].rearrange("(kt p) h -> p kt h", p=128)
                    )
                    b1_sb = const_pool.tile([128, HID], F32)
                    nc.sync.dma_start(b1_sb[:], b1rep[:])
                    b2_sb = const_pool.tile([128, OUT], F32)
                    nc.sync.dma_start(b2_sb[:], b2rep[:])
                    nd_sb = const_pool.tile([128, nb], F32)
                    nc.sync.dma_start(nd_sb[:], nd[:])
                    gidx_sb = const_pool.tile([128, nslot // 16], I16)
                    nc.sync.dma_start(gidx_sb[:], gidx[:])
                    dkey_sb = const_pool.tile([128, nch], BF16)
                    nc.sync.dma_start(dkey_sb[:], dkey[:])
                    iota_sb = const_pool.tile([128, cmax, 128], BF16)
                    nc.sync.dma_start(iota_sb[:], iota3[:])
                    ident_sb = const_pool.tile([128, 128], BF16)
                    make_identity(nc, ident_sb[:])
                    return dict(
                        w2=w2_sb, b1=b1_sb, b2=b2_sb, nd=nd_sb,
                        gidx=gidx_sb, dkey=dkey_sb, iota=iota_sb,
                        ident=ident_sb,
                    )

                cst = None
                for q in range(NQ):
                    y_stage = stage_pool.tile([128, nbq, HID], BF16, tag="yst")
                    for bq in range(nbq):
                        b = q * nbq + bq
                        ypsum = psA.tile([128, HID], F32, tag="agg")
                        for kt in range(KIN):
                            nc.tensor.matmul(
                                ypsum[:],
                                lhsT=xt_sb[kt][:, b * 128 : (b + 1) * 128],
                                rhs=w1_sb[:, kt, :],
                                start=(kt == 0),
                                stop=(kt == KIN - 1),
                            )
                        nc.vector.tensor_tensor(
                            out=y_stage[:, bq, :],
                            in0=ypsum[:],
                            in1=ns_sb[:, b : b + 1].to_broadcast([128, HID]),
                            op=mybir.AluOpType.mult,
                        )
                    nc.sync.dma_start(
                        y_bounce[q], y_stage[:].rearrange("p b h -> p (b h)")
                    )
                    allgather(
                        y_bounce[q], y_full[q * QTOT : (q + 1) * QTOT, :]
                    )
                    if q == 0:
                        cst = late_consts()

            qn = [0]
            GWIN = 8  # 1024-desc calls (ucode cap); ring now fits 2/queue

            with (
                tc.tile_pool(name="msgs", bufs=8) as msgs_pool,
                tc.tile_pool(name="oh", bufs=6) as oh_pool,
            ):

                def mp_layer(table, feat, part_sb, consumer, post_block=None):

                    def run_quarter(s, blk_done):
                        base = table[s * QTOT : (s + 1) * QTOT, :]
                        win_tiles = {}

                        def ensure_win(ch):
                            h0 = q_ch0[s]
                            w0 = h0 + ((ch - h0) // GWIN) * GWIN
                            if w0 not in win_tiles:
                                gw = min(GWIN, h0 + q_nch[s] - w0)
                                t = msgs_pool.tile(
                                    [128, gw, feat], BF16, tag="msgs"
                                )
                                nc.gpsimd.dma_gather(
                                    out_ap=t[:],
                                    in_ap=base,
                                    idxs_ap=cst["gidx"][
                                        :, w0 * 8 : (w0 + gw) * 8
                                    ],
                                    num_idxs=gw * 128,
                                    num_idxs_reg=gw * 128,
                                    elem_size=feat,
                                    queue_num=qn[0] % 4,
                                )
                                qn[0] += 1
                                win_tiles[w0] = t
                            return win_tiles[w0], w0

                        for b in range(nb):
                            cc = int(C[b, s])
                            c0 = int(chunk_off[b, s])
                            oh = oh_pool.tile([128, cc, 128], BF16, tag="oh")
                            nc.vector.tensor_tensor(
                                out=oh[:],
                                in0=cst["iota"][:, :cc, :],
                                in1=cst["dkey"][
                                    :, c0 : c0 + cc, None
                                ].to_broadcast([128, cc, 128]),
                                op=mybir.AluOpType.is_equal,
                            )
                            agg = psA.tile([128, feat], F32, tag="agg")
                            for ci in range(cc):
                                t, w0 = ensure_win(c0 + ci)
                                nc.tensor.matmul(
                                    agg[:],
                                    lhsT=oh[:, ci, :],
                                    rhs=t[:, c0 + ci - w0, :],
                                    start=(ci == 0),
                                    stop=(ci == cc - 1),
                                )
                            blk_done(b, agg)

                    def acc_first(b, agg):
                        nc.scalar.activation(
                            out=part_sb[:, b, :],
                            in_=agg[:],
                            func=mybir.ActivationFunctionType.Copy,
                        )

                    def acc_mid(b, agg):
                        nc.vector.tensor_tensor(
                            out=part_sb[:, b, :],
                            in0=part_sb[:, b, :],
                            in1=agg[:],
                            op=mybir.AluOpType.add,
                        )

                    def acc_last(b, agg):
                        consumer(b, agg)
                        if post_block is not None:
                            post_block(b)

                    run_quarter(0, acc_first)
                    run_quarter(1, acc_mid)
                    run_quarter(2, acc_mid)
                    run_quarter(3, acc_last)

                # ---- L1 consumer: h = relu((agg+part)*nd + b1); z = ns*(h@W2)
                part1 = part_pool.tile([128, nb, HID], BF16, tag="p1")
                z_stages = [None]

                def l1_out(b, agg):
                    q, bq = b // nbq, b % nbq
                    if bq == 0:
                        z_stages[0] = stage_pool.tile(
                            [128, nbq, OUT], BF16, tag="zst", name=f"zst{q}"
                        )
                    t_sb = work_pool.tile([128, HID], F32, tag="tsb")
                    nc.vector.tensor_tensor(
                        out=t_sb[:],
                        in0=agg[:],
                        in1=part1[:, b, :],
                        op=mybir.AluOpType.add,
                    )
                    nc.vector.tensor_tensor(
                        out=t_sb[:],
                        in0=t_sb[:],
                        in1=cst["nd"][:, b : b + 1].to_broadcast([128, HID]),
                        op=mybir.AluOpType.mult,
                    )
                    nc.vector.tensor_tensor(
                        out=t_sb[:],
                        in0=t_sb[:],
                        in1=cst["b1"][:],
                        op=mybir.AluOpType.add,
                    )
                    h_sb = work_pool.tile([128, HID], BF16, tag="hsb")
                    nc.scalar.activation(
                        out=h_sb[:],
                        in_=t_sb[:],
                        func=mybir.ActivationFunctionType.Relu,
                    )
                    hT_sb = work_pool.tile([128, KH, 128], BF16, tag="hT")
                    for ft in range(KH):
                        tp = psB.tile([128, 128], BF16, tag="tr")
                        nc.tensor.transpose(
                            tp[:],
                            h_sb[:, ft * 128 : (ft + 1) * 128],
                            cst["ident"][:],
                        )
                        nc.scalar.activation(
                            out=hT_sb[:, ft, :],
                            in_=tp[:],
                            func=mybir.ActivationFunctionType.Copy,
                        )
                    zp = psC.tile([128, OUT], F32, tag="zp")
                    for kt in range(KH):
                        nc.tensor.matmul(
                            zp[:],
                            lhsT=hT_sb[:, kt, :],
                            rhs=cst["w2"][:, kt, :],
                            start=(kt == 0),
                            stop=(kt == KH - 1),
                        )
                    nc.vector.tensor_tensor(
                        out=z_stages[0][:, bq, :],
                        in0=zp[:],
                        in1=ns_sb[:, b : b + 1].to_broadcast([128, OUT]),
                        op=mybir.AluOpType.mult,
                    )
                    if bq == nbq - 1:
                        nc.sync.dma_start(
                            z_bounce[q],
                            z_stages[0][:].rearrange("p b h -> p (b h)"),
                        )
                        allgather(
                            z_bounce[q], z_full[q * QTOT : (q + 1) * QTOT, :]
                        )

                mp_layer(y_full, HID, part1, l1_out)

                # ---- L2 consumer: out = (agg+part)*nd + b2 ----
                part2 = part_pool.tile([128, nb, OUT], BF16, tag="p2")
                out_sb = part_pool.tile([128, nb, OUT], F32, tag="osb")

                def l2_out(b, agg):
                    nc.vector.tensor_tensor(
                        out=out_sb[:, b, :],
                        in0=agg[:],
                        in1=part2[:, b, :],
                        op=mybir.AluOpType.add,
                    )
                    nc.vector.tensor_tensor(
                        out=out_sb[:, b, :],
                        in0=out_sb[:, b, :],
                        in1=cst["nd"][:, b : b + 1].to_broadcast([128, OUT]),
                        op=mybir.AluOpType.mult,
                    )
                    nc.vector.tensor_tensor(
                        out=out_sb[:, b, :],
                        in0=out_sb[:, b, :],
                        in1=cst["b2"][:],
                        op=mybir.AluOpType.add,
                    )

                mp_layer(z_full, OUT, part2, l2_out)
                nc.sync.dma_start(out_t[:], out_sb[:])

    nc.compile()
    return nc


# ----------------------------------------------------------------------------
# Entry point
# ----------------------------------------------------------------------------

def assemble_output(results, meta):
    NPC, OUT, nb = meta["NPC"], meta["OUT"], meta["nb"]
    full = np.empty((meta["NTOT"], OUT), dtype=np.float32)
    for c, r in enumerate(results):
        o = np.asarray(r["out"])  # [128, nb, OUT]
        full[c * NPC : (c + 1) * NPC] = o.transpose(1, 0, 2).reshape(NPC, OUT)
    return np.ascontiguousarray(full[meta["pos"]])


def kernel(x, W1, b1, W2, b2, src, dst, _run=None, **_kw):
    x = np.asarray(x, dtype=np.float32)
    W1 = np.asarray(W1, dtype=np.float32)
    W2 = np.asarray(W2, dtype=np.float32)
    b1 = np.asarray(b1, dtype=np.float32)
    b2 = np.asarray(b2, dtype=np.float32)
    src = np.asarray(src)
    dst = np.asarray(dst)

    in_maps, meta = prepare(x, W1, b1, W2, b2, src, dst, ncores=NCORES)
    nc = build_nc(meta)

    if _run is None:
        res = run_bass_kernel_spmd(nc, in_maps, core_ids=list(range(meta["ncores"])))
        results = res.results
    else:
        results = _run(nc, in_maps)

    return assemble_output(results, meta)
